# revision 139
# baseline (speedup 1.0000x reference)
"""SAGAN-style self-attention block on 8 trn2 NeuronCores.

Full inputs: x [8, 512, 64, 64], w_theta [64, 512], w_phi [64, 512],
w_g [256, 512], w_o [512, 256], gamma scalar.

Sharding: data-parallel over batch — one batch item per core. Each core runs
an identical Bass program over its own x[b]; weights are replicated.

Per-core math (C=512, n=H*W=4096, m=n/4=1024):
  theta = w_theta @ x            [64, 4096]
  phi   = pool2(w_phi @ x)       [64, 1024]
  g     = pool2(w_g @ x)         [256, 1024]   (bf16)
  S^T   = phi^T @ theta          [1024, 4096]  (scores, transposed layout)
  E     = exp(S^T)               (bf16; no max-subtraction: |S| < ~50)
  Z     = colsum(E)              progressive adder tree (DVE pairs + Pool
                                 partials) + one broadcast ones-matmul
  att   = (g @ E) / Z            [256, 4096]
  out   = (gamma*w_o) @ att + x  [512, 4096]

Precision: x and the projection weights load as bf16 (halves the input DMA
stream that paces phase 1); scores/attend/output matmuls run at the PE's
full rate either way (bf16 and float32r are both 1 cycle/row). The output
is stored bf16 and widened on the host. Measured rel err 3.4e-3 vs the
fp32 reference (budget 2e-2); the softmax normalization cancels most of
the correlated quantization error in the attention weights.

Schedule: phase 1 (projections, pools, transposes) is PE-paced at ~3.1us
per 512-column slice with slice-0/1 score matmuls riding along; the x
chunks stream in over three DMA queues (sync/HWDGE + gpsimd/SWDGE) to keep
descriptor generation off the critical path. Phase 2 runs one attend slice
per ~7.0us iteration — exactly its PE work, zero steady-state stall — with
scores prefetched two slices ahead and the softmax-normalizer tree (DVE
pair adds, Pool partial sums, one broadcast ones-matmul, DVE reciprocal)
one slice ahead. The final slice is split into two 256-column chunks whose
residual is an identity matmul accumulated in psum and evacuated by
ACT/DVE copies, keeping the tail off the busy vector engine.
"""

import time
from contextlib import ExitStack

import numpy as np

import bass_rust
import concourse.bass as bass
import concourse.mybir as mybir
import concourse.tile as tile
from concourse.bass_utils import run_bass_kernel_spmd
from concourse.masks import make_identity

P = 128
C = 512  # channels
C8 = 64  # theta/phi channels
C2 = 256  # g channels
N = 4096  # H*W
M = 1024  # pooled spatial
NS = 8  # n-slices
SL = 512  # n-slice width
MT = 8  # m-tiles of 128
F32 = mybir.dt.float32
F32R = mybir.dt.float32r
BF16 = mybir.dt.bfloat16
AX = mybir.AxisListType
ALU = mybir.AluOpType
ACTF = mybir.ActivationFunctionType


def _pool_view(ap):
    """[p, 512] slice of the conv output -> 5D maxpool view [p, h2, w2, dy, dx].

    Within an n-slice of 512 = 8 image rows: local n = (2*h2+dy)*64 + 2*w2+dx.
    """
    return ap.rearrange("p (h2 dy w2 dx) -> p h2 w2 dy dx", h2=4, dy=2, w2=32, dx=2)


def emit(nc, tc, ctx):
    ctx.enter_context(nc.allow_low_precision(reason="bf16 attend path"))
    x_f = nc.dram_tensor("x", [C, N], BF16, kind="ExternalInput")
    wproj = nc.dram_tensor("wproj", [C, 384], BF16, kind="ExternalInput")
    wo = nc.dram_tensor("wo", [C2, C], F32R, kind="ExternalInput")
    out_d = nc.dram_tensor("out", [C, N], BF16, kind="ExternalOutput")

    persist = ctx.enter_context(tc.tile_pool(name="persist", bufs=1))

    ident_f = persist.tile([P, P], F32)
    make_identity(nc, ident_f)
    ident = persist.tile([P, P], BF16)
    nc.vector.tensor_copy(ident, ident_f)
    ident_r = persist.tile([P, P], F32R)
    nc.vector.tensor_copy(ident_r, ident_f)
    ones = persist.tile([P, P], BF16)
    nc.vector.memset(ones, 1.0)

    # score psum pool lives across phases 1+2 so slice-0 scores can start
    # inside phase 1
    spool = ctx.enter_context(tc.tile_pool(name="spsum", bufs=2, space="PSUM"))
    etp = ctx.enter_context(tc.tile_pool(name="et", bufs=5))
    miscp = ctx.enter_context(tc.tile_pool(name="misc", bufs=2))

    # Warm-up matmuls for the otherwise-idle startup window: ramp the PE
    # clock (p-state) on constant data before real work arrives (~4.5us in,
    # once the first x chunk + first wproj chunk land).
    for wi in range(6):
        wt_ = spool.tile([P, P], F32, name="warm", tag=f"s{wi % 2}", bufs=1)
        nc.tensor.matmul(wt_, lhsT=ident_f, rhs=ident_f, start=True, stop=True)

    # Loads. The DMA engine pool is a serial ~360GB/s resource, so issue
    # order is arrival order: interleave the wproj chunks with slice-0's x
    # chunks so the slice-0 k-loop can start as data lands. One x chunk per
    # slice goes through the gpsimd (SWDGE) queue: HWDGE's fixed 625ns
    # descriptor-generation per DMA would otherwise pace the bf16 stream.
    wpt = persist.tile([P, 4, 384], BF16, name="wpt")
    wpsrc = wproj[:, :].rearrange("(k p) o -> p k o", k=4)
    xft = persist.tile([P, 4, N], BF16, name="xft")
    xsrc = x_f[:, :].rearrange("(k p) n -> p k n", k=4)
    wott = persist.tile([P, 2, C], F32R, name="wott")

    for k in range(4):
        nc.scalar.dma_start(out=wpt[:, k, :], in_=wpsrc[:, k, :])
        nc.sync.dma_start(out=xft[:, k, 0:SL], in_=xsrc[:, k, 0:SL])
    for q in range(1, NS):
        for cc in range(4):
            eng = nc.gpsimd if cc == 0 else nc.sync
            eng.dma_start(
                out=xft[:, cc, q * SL : (q + 1) * SL],
                in_=xsrc[:, cc, q * SL : (q + 1) * SL],
            )
        if q == 2:
            nc.gpsimd.dma_start(
                out=wott, in_=wo[:, :].rearrange("(k p) o -> p k o", k=2)
            )
    wp = [wpt[:, k, :] for k in range(4)]
    xf = [xft[:, k, :] for k in range(4)]
    wot = [wott[:, k, :] for k in range(2)]

    theta = persist.tile([C8, N], F32R)
    phi = persist.tile([P, M], F32R)  # [64:128] pooled, [0:64] copy for scores
    g = [persist.tile([P, M], BF16, name=f"g{i}") for i in range(2)]
    gT = [persist.tile([P, C2], BF16, name=f"gT{mt}") for mt in range(MT)]

    ET = [[None] * MT for _ in range(NS)]
    TPAIR = [[None] * (MT // 2) for _ in range(NS)]
    RINV = [None] * NS
    ZH = [None] * NS

    def emit_score_one(i, mt):
        # one m-tile of slice i's scores + exp; pair-add for the Z tree once
        # the odd member of a pair exists (bf16 2x mode, off critical path).
        # 3-bank ring so the matmuls run up to 3 exps ahead of ACT.
        nsl = slice(i * SL, (i + 1) * SL)
        sp = spool.tile([P, SL], F32, name="sp", tag=f"s{mt % 2}", bufs=1)
        nc.tensor.matmul(
            sp,
            lhsT=phi[0:C8, mt * P : (mt + 1) * P],
            rhs=theta[:, nsl],
            start=True,
            stop=True,
        )
        et = etp.tile([P, SL], BF16, name="et", tag=f"et{mt}")
        nc.scalar.activation(et, sp, ACTF.Exp)
        ET[i][mt] = et
        if mt % 2 == 1:
            j = mt // 2
            tp_ = miscp.tile([P, SL], BF16, name="tpair", tag=f"tp{j}", bufs=2)
            nc.vector.tensor_add(tp_, ET[i][mt - 1], ET[i][mt])
            TPAIR[i][j] = tp_

    def make_zp(i):
        # progressive tree: two partial sums on Pool early, final add on DVE
        # right after the last pair lands — short latency from last exp to ZH
        s1 = miscp.tile([P, SL], BF16, name="zq", tag="zq0", bufs=2)
        nc.gpsimd.tensor_add(s1, TPAIR[i][0], TPAIR[i][1])
        s2 = miscp.tile([P, SL], BF16, name="zq", tag="zq1", bufs=2)
        nc.gpsimd.tensor_add(s2, s1, TPAIR[i][2])
        h = miscp.tile([P, SL], BF16, name="zh", tag="zh", bufs=2)
        nc.vector.tensor_add(h, s2, TPAIR[i][3])
        ZH[i] = h

    # ---- phase 1: projections + pooling + g transposes -----------------
    with tc.tile_pool(name="ppsum", bufs=2, space="PSUM") as pp, tc.tile_pool(
        name="tpsum", bufs=1, space="PSUM"
    ) as tp:
        def emit_transpose(ns, i):
            # transpose slice ns's pooled g columns into gT[ns] (bf16);
            # called one slice late so the pools are long done. DVE evacuates
            # the psum (192ns) — keeps ACT free for score exps.
            msl = slice(ns * P, (ns + 1) * P)
            t = tp.tile([P, P], BF16, name="tp", tag="tp")
            nc.tensor.transpose(t, g[i][:, msl], ident)
            nc.scalar.copy(out=gT[ns][:, i * P : (i + 1) * P], in_=t)

        for ns in range(NS):
            nsl = slice(ns * SL, (ns + 1) * SL)
            msl = slice(ns * P, (ns + 1) * P)
            xr = [xf[k][:, nsl] for k in range(4)]
            ps = [
                pp.tile(
                    [P, SL], F32, name="pp", tag=f"pp{mt}",
                    bufs=(1 if mt == 0 else 2),
                )
                for mt in range(3)
            ]
            if ns == 0:
                # k-outer so the first matmuls need only the first wproj +
                # x chunks — starts the PE ~2us earlier
                for k in range(4):
                    for mt in (0, 1, 2):
                        nc.tensor.matmul(
                            ps[mt],
                            lhsT=wp[k][:, mt * P : (mt + 1) * P],
                            rhs=xr[k],
                            start=(k == 0),
                            stop=(k == 3),
                            skip_group_check=True,
                        )
            else:
                if ns > 1:
                    emit_transpose(ns - 2, 1)
                for mt in (1, 2, 0):
                    for k in range(4):
                        nc.tensor.matmul(
                            ps[mt],
                            lhsT=wp[k][:, mt * P : (mt + 1) * P],
                            rhs=xr[k],
                            start=(k == 0),
                            stop=(k == 3),
                        )
            # g pools first: with the g-first matmul order their psums are
            # ready first
            for i in range(2):
                nc.vector.tensor_reduce(
                    out=g[i][:, msl],
                    in_=_pool_view(ps[1 + i]),
                    axis=AX.XY,
                    op=ALU.max,
                )
            nc.vector.tensor_reduce(
                out=phi[C8:P, msl],
                in_=_pool_view(ps[0][C8:P, :]),
                axis=AX.XY,
                op=ALU.max,
            )
            # early slices: sync queue is clogged by the x stream; late
            # slices: the Pool queue is clogged by the Z-tree partial sums
            peng = nc.sync if ns >= 3 else nc.gpsimd
            peng.dma_start(out=phi[0:C8, msl], in_=phi[C8:P, msl])
            if ns == NS - 1:
                # DVE copy: ACT is busy with score exps here, and the psum
                # bank release gates phase-2's first attend matmuls
                nc.vector.tensor_copy(out=theta[:, nsl], in_=ps[0][0:C8, :])
            else:
                nc.scalar.copy(out=theta[:, nsl], in_=ps[0][0:C8, :])
            if ns > 0:
                with tc.high_priority(offset=6):
                    emit_transpose(ns - 1, 0)
                emit_score_one(0, ns - 1)
            if ns > 1:
                emit_score_one(1, ns - 2)
        with tc.high_priority(offset=6):
            emit_transpose(NS - 2, 1)
        emit_score_one(0, NS - 1)
        emit_transpose(NS - 1, 0)
        emit_score_one(1, NS - 2)
        emit_transpose(NS - 1, 1)
        emit_score_one(1, NS - 1)
        make_zp(0)

    # ---- phase 2: softmax / attend / project ---------------------------
    with tc.tile_pool(name="qpsum", bufs=2, space="PSUM") as qp:
        def emit_scores(i):
            for mt in range(MT):
                emit_score_one(i, mt)

        def emit_zfin(i):
            # broadcast ones-matmul finishes the partition sum of the tree
            zp = qp.tile([P, SL], F32, name="zp", tag="z", bufs=1)
            nc.tensor.matmul(zp, lhsT=ones, rhs=ZH[i], start=True, stop=True)
            rinv = miscp.tile([P, SL], F32, name="rinv", tag="rinv", bufs=2)
            nc.vector.reciprocal(rinv, zp)
            RINV[i] = rinv

        def emit_ap(i, lo, w):
            # attend accumulation for columns [i*SL+lo, i*SL+lo+w)
            esl = slice(lo, lo + w)
            ap = [qp.tile([P, w], F32, name="ap", tag="a", bufs=2) for _ in range(2)]
            for mt in range(MT):
                st, sp_ = (mt == 0), (mt == MT - 1)
                for ct in range(2):
                    nc.tensor.matmul(
                        ap[ct],
                        lhsT=gT[mt][:, ct * P : (ct + 1) * P],
                        rhs=ET[i][mt][:, esl],
                        start=st,
                        stop=sp_,
                        skip_group_check=True,
                    )
            return ap

        def emit_att(i, lo, w, ap):
            # normalize: att = ap / Z
            esl = slice(lo, lo + w)
            att = []
            for ct in range(2):
                t = miscp.tile([P, w], F32R, name="att", tag=f"att{ct}")
                nc.vector.tensor_mul(t, ap[ct], RINV[i][:, esl])
                att.append(t)
            return att

        def emit_proj_store(i, lo, w, att, split_store=False, tail=False):
            # project + residual + store. In tail mode the residual is an
            # identity matmul accumulated into the psum and ACT/DVE evacuate
            # it — keeps the busy DVE off the critical tail.
            nsl = slice(i * SL + lo, i * SL + lo + w)
            ob = miscp.tile([P, 4, w], BF16, name="ob", tag=f"ob{(lo // 256) % 2}")
            odst = out_d[:, :].rearrange("(k p) n -> p k n", k=4)
            for ot in range(4):
                if tail and ot == 1:
                    ttag, tbufs = "z", 1
                elif tail and ot == 3:
                    ttag, tbufs = "a", 2
                else:
                    ttag, tbufs = "o", 2
                op_ = qp.tile([P, w], F32, name="op", tag=ttag, bufs=tbufs)
                for ct in range(2):
                    nc.tensor.matmul(
                        op_,
                        lhsT=wot[ct][:, ot * P : (ot + 1) * P],
                        rhs=att[ct],
                        start=(ct == 0),
                        stop=(ct == 1) and not tail,
                    )
                if tail:
                    nc.tensor.matmul(
                        op_,
                        lhsT=ident,
                        rhs=xf[ot][:, nsl],
                        start=False,
                        stop=True,
                    )
                    if ot % 2 == 0:
                        nc.scalar.copy(out=ob[:, ot, :], in_=op_)
                    else:
                        nc.vector.tensor_copy(ob[:, ot, :], op_)
                else:
                    nc.vector.tensor_add(ob[:, ot, :], op_, xf[ot][:, nsl])
                if split_store and ot == 2:
                    nc.sync.dma_start(out=odst[:, 0:3, nsl], in_=ob[:, 0:3, :])
                elif split_store and ot == 3:
                    nc.sync.dma_start(out=odst[:, 3, nsl], in_=ob[:, 3, :])
            if not split_store:
                nc.sync.dma_start(out=odst[:, :, nsl], in_=ob)

        for i in range(NS - 1):
            if i == 0:
                emit_scores(2)
                # scheduler hint: let the attend matmuls overtake the
                # transition score burst (which is exp/bank-gated) in the
                # PE queue
                with tc.high_priority(offset=10):
                    ap = emit_ap(i, 0, SL)
                emit_zfin(0)
            else:
                ap = emit_ap(i, 0, SL)
            if 0 < i and i + 2 < NS:
                emit_scores(i + 2)
            if i == NS - 2:
                # interleave the last slice's attend/normalize so the PE and
                # DVE stay busy while slice-6's output chain drains
                ap0 = emit_ap(NS - 1, 0, 256)
                att = emit_att(i, 0, SL, ap)
                emit_proj_store(i, 0, SL, att, split_store=True)
                make_zp(NS - 1)
                emit_zfin(NS - 1)
                att0 = emit_att(NS - 1, 0, 256, ap0)
            else:
                att = emit_att(i, 0, SL, ap)
                emit_proj_store(i, 0, SL, att)
                make_zp(i + 1)
                emit_zfin(i + 1)
        # last slice: two 256-wide chunks; chunk-1's output chain is the
        # kernel tail, keep it short (identity-residual matmuls, ACT/DVE
        # psum evacuation, grouped stores)
        ap1 = emit_ap(NS - 1, 256, 256)
        emit_proj_store(NS - 1, 0, 256, att0, tail=True)
        att1 = emit_att(NS - 1, 256, 256, ap1)
        emit_proj_store(NS - 1, 256, 256, att1, tail=True, split_store=True)


def build_nc():
    nc = bass.Bass(target_bir_lowering=False, trn_type="TRN2")
    with tile.TileContext(nc) as tc:
        with ExitStack() as ctx:
            emit(nc, tc, ctx)
    bass_rust.generate_event_semaphores(nc)
    return nc


def kernel(x, w_theta, w_phi, w_g, w_o, gamma):
    x = np.asarray(x, dtype=np.float32)
    B = x.shape[0]
    wproj = np.ascontiguousarray(
        np.concatenate(
            [np.asarray(w_theta).T, np.asarray(w_phi).T, np.asarray(w_g).T], axis=1
        ),
        dtype=np.float32,
    )
    wo_t = np.ascontiguousarray(
        (np.float32(gamma) * np.asarray(w_o)).T, dtype=np.float32
    )

    import ml_dtypes

    bf16 = ml_dtypes.bfloat16
    wproj = wproj.astype(bf16)
    nc = build_nc()
    in_maps = []
    for b in range(B):
        xb = np.ascontiguousarray(x[b].reshape(C, N)).astype(bf16)
        in_maps.append({"x": xb, "wproj": wproj, "wo": wo_t})
    # retry: rare transient NRT_EXEC_UNIT_UNRECOVERABLE from stale device
    # state clears on re-execution
    last_err = None
    for attempt in range(3):
        try:
            res = run_bass_kernel_spmd(nc, in_maps, core_ids=list(range(B)))
            break
        except Exception as e:  # noqa: BLE001
            last_err = e
            time.sleep(2.0)
    else:
        raise last_err
    out = np.stack(
        [np.asarray(res.results[b]["out"]).astype(np.float32).reshape(C, 64, 64)
         for b in range(B)]
    )
    return out
